# revision 2
# baseline (speedup 1.0000x reference)
"""Trainium2 Bass kernel for nn_BiologicalRNN_88210038326490.

Reference semantics (jax):
    input_mask  : W_hi rows [0, H/2) active
    output_mask : W_oh cols [H/2, H) active
    k = dt/tau = 0.05
    h_{t+1} = h_t + k*(-h_t + x_t @ W_hi_m.T + elu(h_t) @ W_hh.T + b)
    out_t   = h_{t+1} @ W_oh_m.T
    returns (outs [B,T,O], h_final [B,H])

Strategy (8 cores, data-parallel over batch, BS=32 per core):
  - State kept transposed on-chip: hT[p, c*BS+b] = h[b, c*128+p]  (c = H-chunk).
  - Recurrent matmul with W tiles stationary (fp16, fast-weight-load) and
    eluT chunks [128, BS] as the moving operand; output lands directly in
    hT layout, so no per-step transpose is ever needed.
  - elu(h) is fed to the matmul as (relu(h) + min(exp(h), 1)) and the "-1"
    is folded into a bias row: psum = W'@(r+e) + (k*b - rowsum(W'))*1 + k*W_hi_m@x_t
    with W' = k*W_hh.  The x/bias term is just a 19-row augmented
    contraction chunk of the same accumulation group.
  - h update in one fused DVE op: h = (h * (1-k)) + psum.
  - Out-projection from an fp16 copy of the second half of h.
"""

import os
import numpy as np
import ml_dtypes

import concourse.bass as bass
import concourse.bacc as bacc
import concourse.tile as tile
from concourse import mybir
from concourse.bass_utils import run_bass_kernel_spmd

f32 = mybir.dt.float32
f16 = mybir.dt.float16

B, T_FULL, H, O, F = 256, 1000, 1024, 16, 18
NCORES = 8
BS = B // NCORES          # 32 rows of the batch per core
C = H // 128              # 8 H-chunks
C2 = C // 2               # chunks in the second half (out-projection)
FAUG = F + 1              # x rows + constant-one row
K_LEAK = 0.05             # dt/tau

_MODULE_CACHE = {}


def _build_module(T, blk):
    """Build + compile the per-core Bass module. blk = steps per For_i body."""
    assert T % blk == 0 and blk % 2 == 0
    nit = T // blk

    nc = bacc.Bacc("TRN2", target_bir_lowering=False)

    wstk = nc.dram_tensor("wstk", [128, C * C * 128], f16, kind="ExternalInput")
    waug = nc.dram_tensor("waug", [FAUG, C * 128], f16, kind="ExternalInput")
    xaug = nc.dram_tensor("xaug", [FAUG, T * BS], f16, kind="ExternalInput")
    woh = nc.dram_tensor("woh", [128, C2 * O], f16, kind="ExternalInput")
    h0 = nc.dram_tensor("h0", [128, C * BS], f32, kind="ExternalInput")
    outsd = nc.dram_tensor("outsd", [O, T * BS], f16, kind="ExternalOutput")
    hfd = nc.dram_tensor("hfd", [128, C * BS], f32, kind="ExternalOutput")

    import contextlib

    with tile.TileContext(nc) as tc, contextlib.ExitStack() as ctx:
        const = ctx.enter_context(tc.tile_pool(name="const", bufs=1))
        psum_p = ctx.enter_context(tc.tile_pool(name="psum", bufs=2, space="PSUM"))
        psum_o = ctx.enter_context(tc.tile_pool(name="psum_o", bufs=2, space="PSUM"))
        tmp = ctx.enter_context(tc.tile_pool(name="tmp", bufs=4))

        w_sb = const.tile([128, C * C * 128], f16)
        waug_sb = const.tile([FAUG, C * 128], f16)
        x_sb = const.tile([FAUG, T * BS], f16)
        woh_sb = const.tile([128, C2 * O], f16)
        h_sb = const.tile([128, C * BS], f32)
        eluA = const.tile([128, C * BS], f16)
        eluB = const.tile([128, C * BS], f16)
        h2f = const.tile([128, C2 * BS], f16)
        outs_sb = const.tile([O, T * BS], f16)

        nc.sync.dma_start(out=w_sb, in_=wstk[:])
        nc.sync.dma_start(out=waug_sb, in_=waug[:])
        nc.sync.dma_start(out=x_sb, in_=xaug[:])
        nc.sync.dma_start(out=woh_sb, in_=woh[:])
        nc.sync.dma_start(out=h_sb, in_=h0[:])

        NG = 4                      # vector ops work on NG groups of C/NG chunks
        GW = (C // NG) * BS         # group width in columns (64)

        def elu_of_h(elu_out):
            """elu_out = relu(h) + min(exp(h),1)   (== elu(h)+1, fp16)."""
            for g in range(NG):
                sl = bass.ts(g, GW)
                e = tmp.tile([128, GW], f16, tag="e")
                nc.scalar.activation(e, h_sb[:, sl], mybir.ActivationFunctionType.Exp)
                em = tmp.tile([128, GW], f16, tag="em")
                nc.gpsimd.tensor_scalar_min(em, e, 1.0)
                nc.vector.scalar_tensor_tensor(
                    out=elu_out[:, sl], in0=h_sb[:, sl], scalar=0.0, in1=em,
                    op0=mybir.AluOpType.max, op1=mybir.AluOpType.add,
                )

        def step(u, elu_in, elu_out):
            """One time step. u: step index (python int or ScalarValue expr)."""
            ps = psum_p.tile([128, C * BS], f32, tag="ps")
            xs = x_sb[:, bass.ts(u, BS)]
            for i in range(C):
                out_sl = ps[:, bass.ts(i, BS)]
                for j in range(C):
                    nc.tensor.matmul(
                        out_sl,
                        w_sb[:, bass.ts(i * C + j, 128)],
                        elu_in[:, bass.ts(j, BS)],
                        start=(j == 0), stop=False,
                    )
                nc.tensor.matmul(
                    out_sl, waug_sb[:, bass.ts(i, 128)], xs,
                    start=False, stop=True,
                )
            # per-group state update + elu for next step
            for g in range(NG):
                sl = bass.ts(g, GW)
                nc.vector.scalar_tensor_tensor(
                    out=h_sb[:, sl], in0=h_sb[:, sl], scalar=1.0 - K_LEAK,
                    in1=ps[:, sl],
                    op0=mybir.AluOpType.mult, op1=mybir.AluOpType.add,
                )
                e = tmp.tile([128, GW], f16, tag="e")
                nc.scalar.activation(e, h_sb[:, sl], mybir.ActivationFunctionType.Exp)
                em = tmp.tile([128, GW], f16, tag="em")
                nc.gpsimd.tensor_scalar_min(em, e, 1.0)
                nc.vector.scalar_tensor_tensor(
                    out=elu_out[:, sl], in0=h_sb[:, sl], scalar=0.0, in1=em,
                    op0=mybir.AluOpType.max, op1=mybir.AluOpType.add,
                )
            # out-projection: out_t = h2 @ W_oh2.T  (h2 = second half of h)
            nc.scalar.copy(h2f, h_sb[:, C2 * BS:])
            po = psum_o.tile([O, BS], f32, tag="po")
            for c4 in range(C2):
                nc.tensor.matmul(
                    po, woh_sb[:, bass.ts(c4, O)], h2f[:, bass.ts(c4, BS)],
                    start=(c4 == 0), stop=(c4 == C2 - 1),
                )
            nc.vector.tensor_copy(outs_sb[:, bass.ts(u, BS)], po)

        elu_of_h(eluA)

        if nit > 1:
            with tc.For_i(0, nit, hint_engines=tuple(mybir.ALL_ENGINES)) as it:
                base = it * blk
                for s in range(blk):
                    src, dst = (eluA, eluB) if s % 2 == 0 else (eluB, eluA)
                    step(base + s, src, dst)
        else:
            for s in range(blk):
                src, dst = (eluA, eluB) if s % 2 == 0 else (eluB, eluA)
                step(s, src, dst)

        nc.sync.dma_start(out=hfd[:], in_=h_sb)
        nc.sync.dma_start(out=outsd[:], in_=outs_sb)

    nc.compile()
    return nc


def _get_module(T, blk):
    key = (T, blk)
    if key not in _MODULE_CACHE:
        _MODULE_CACHE[key] = _build_module(T, blk)
    return _MODULE_CACHE[key]


def _prep_core_inputs(x_core, W_hh, W_hi, W_oh, b, hidden_init, T):
    """Host-side packing for one core. x_core: [BS, T, F] float32."""
    k = np.float32(K_LEAK)
    Wp = (k * W_hh).astype(np.float16)               # [H, H] fp16 (k folded)
    # wstk[p, (i*C+j)*128 + m] = Wp[i*128+m, j*128+p]
    W4 = Wp.reshape(C, 128, C, 128)                  # [i, m, j, p]
    wstk = np.ascontiguousarray(W4.transpose(3, 0, 2, 1).reshape(128, C * C * 128))

    rowsum = Wp.astype(np.float32).sum(axis=1)       # [H] rowsum of fp16 weights
    W_him = W_hi.copy()
    W_him[H // 2:, :] = 0.0
    waug = np.zeros((FAUG, C * 128), np.float16)
    waug[:F, :] = (k * W_him).T.astype(np.float16)   # [F, H]
    waug[F, :] = (k * b - rowsum).astype(np.float16)

    xa = np.ones((FAUG, T, BS), np.float32)
    xa[:F] = x_core.transpose(2, 1, 0)               # [F, T, BS]
    xaug = xa.reshape(FAUG, T * BS).astype(np.float16)

    # woh[p, c4*O + o] = W_oh[o, (C2+c4)*128 + p]
    Woh2 = W_oh[:, H // 2:].astype(np.float16)       # [O, H/2]
    woh = np.ascontiguousarray(
        Woh2.reshape(O, C2, 128).transpose(2, 1, 0).reshape(128, C2 * O))

    # h0T[p, c*BS + b] = hidden_init[c*128+p]
    h0 = np.repeat(hidden_init.reshape(C, 128).T[:, :, None], BS, axis=2)
    h0 = np.ascontiguousarray(h0.reshape(128, C * BS).astype(np.float32))

    return {"wstk": wstk, "waug": waug, "xaug": xaug, "woh": woh, "h0": h0}


def run(x, W_hi, W_hh, b, W_oh, hidden_init, T=T_FULL, blk=20, trace=False):
    x = np.asarray(x, np.float32)
    W_hi = np.asarray(W_hi, np.float32)
    W_hh = np.asarray(W_hh, np.float32)
    b = np.asarray(b, np.float32)
    W_oh = np.asarray(W_oh, np.float32)
    hidden_init = np.asarray(hidden_init, np.float32)

    nc = _get_module(T, blk)
    in_maps = [
        _prep_core_inputs(x[c * BS:(c + 1) * BS, :T], W_hh, W_hi, W_oh, b,
                          hidden_init, T)
        for c in range(NCORES)
    ]
    res = run_bass_kernel_spmd(nc, in_maps, core_ids=list(range(NCORES)),
                               trace=trace)

    outs = np.empty((B, T, O), np.float32)
    h_final = np.empty((B, H), np.float32)
    for c in range(NCORES):
        om = res.results[c]["outsd"].astype(np.float32)   # [O, T*BS]
        outs[c * BS:(c + 1) * BS] = (
            om.reshape(O, T, BS).transpose(2, 1, 0))
        hf = res.results[c]["hfd"]                        # [128, C*BS]
        h_final[c * BS:(c + 1) * BS] = (
            hf.reshape(128, C, BS).transpose(2, 1, 0).reshape(BS, H))
    return (outs, h_final), res


def kernel(x, W_hi, W_hh, b, W_oh, hidden_init):
    (outs, h_final), _ = run(x, W_hi, W_hh, b, W_oh, hidden_init)
    return outs, h_final


# revision 56
# speedup vs baseline: 407.6840x; 407.6840x over previous
"""Trainium2 Bass kernel for nn_BiologicalRNN_88210038326490.

Reference semantics (jax):
    input_mask  : W_hi rows [0, H/2) active
    output_mask : W_oh cols [H/2, H) active
    k = dt/tau = 0.05
    h_{t+1} = h_t + k*(-h_t + x_t @ W_hi_m.T + elu(h_t) @ W_hh.T + b)
    out_t   = h_{t+1} @ W_oh_m.T
    returns (outs [B,T,O], h_final [B,H])

Strategy (8 cores, data-parallel over batch, BS=32 per core):
  - State kept transposed on-chip: hT[p, c*BS+b] = h[b, c*128+p]  (c = H-chunk).
  - Recurrent matmul with W tiles stationary (fp16, fast-weight-load) and
    eluT chunks [128, BS] as the moving operand; output lands directly in
    hT layout, so no per-step transpose is ever needed.
  - elu(h) = relu(h) - relu(1 - exp(h)) exactly (incl. fp16 exp overflow ->
    inf -> relu(-inf)=0), so the elu feed costs one ACT Exp, one ACT Relu
    (with scale=-1, bias=1) and one fused DVE op; the x/bias term
    k*(W_hi_m@x_t + b) is a 19-row augmented contraction chunk of the same
    accumulation group (x rows + a constant-one row).
  - h update in one fused DVE op: h = (h * (1-k)) + psum.
  - State/elu/psum are split into NG per-group tiles so Tile's tile-granular
    dependency tracking lets each group's vector chain overlap the other
    groups' matmuls (single shared tiles serialize PE vs vector entirely).
"""

import numpy as np

import concourse.bass as bass
import concourse.bacc as bacc
import concourse.tile as tile
from concourse import mybir
from concourse.bass_utils import run_bass_kernel_spmd

f32 = mybir.dt.float32
f16 = mybir.dt.float16

B, T_FULL, H, O, F = 256, 1000, 1024, 16, 18
NCORES = 8
BS = B // NCORES          # 32 rows of the batch per core
C = H // 128              # 8 H-chunks
C2 = C // 2               # chunks in the second half (out-projection)
FAUG = F + 1              # x rows + constant-one row
K_LEAK = 0.05             # dt/tau

# Recurrent-weight dtype options: name -> (mybir dtype, scale S).
# For fp8, W is stored as S*k*W_hh and the elu feed is produced as elu(h)/S
# (exp bias fold + h/S shadow state), so psum comes out at true scale.
_WDT = {
    "f16": (f16, 1.0),
    "e4": (mybir.dt.float8e4, 8192.0),
    "e3": (mybir.dt.float8e3, 1024.0),
}

_MODULE_CACHE = {}


def _build_module(T, blk, reps=1, ng=4, out_eng="act", staggered=False,
                  wdt="f16", kaug=128, po16=True, hint="none", tmpbufs=8,
                  pstail=2, ntail=1):
    """Build + compile the per-core Bass module. blk = steps per For_i body."""
    assert T % blk == 0 and blk % 2 == 0
    nit = T // blk
    NG = ng                   # state groups (C/NG chunks each)
    CPG = C // NG             # chunks per group
    GW = CPG * BS             # group width in columns
    w_dt, S = _WDT[wdt]
    import math
    exp_bias = -math.log(S)   # exp(h)/S == exp(h + exp_bias)

    nc = bacc.Bacc("TRN2", target_bir_lowering=False)

    wstk = nc.dram_tensor("wstk", [128, C * C * 128], w_dt, kind="ExternalInput")
    waug = nc.dram_tensor("waug", [kaug, C * 128], f16, kind="ExternalInput")
    xaug = nc.dram_tensor("xaug", [kaug, T * BS], f16, kind="ExternalInput")
    woh = nc.dram_tensor("woh", [128, C2 * O], f16 if po16 else f32,
                         kind="ExternalInput")
    h0 = nc.dram_tensor("h0", [128, C * BS], f32, kind="ExternalInput")
    OLAG = 2 if po16 else 1
    outsd = nc.dram_tensor("outsd", [O, (T + OLAG) * BS], f16,
                           kind="ExternalOutput")
    hfd = nc.dram_tensor("hfd", [128, C * BS], f32, kind="ExternalOutput")

    import contextlib

    with tile.TileContext(nc) as tc, contextlib.ExitStack() as ctx:
        const = ctx.enter_context(tc.tile_pool(name="const", bufs=1))
        psum_p = ctx.enter_context(
            tc.tile_pool(name="psum", bufs=(1 if NG >= 4 else 2), space="PSUM"))
        psum_o = ctx.enter_context(tc.tile_pool(name="psum_o", bufs=2, space="PSUM"))
        tmp = ctx.enter_context(tc.tile_pool(name="tmp", bufs=tmpbufs))

        w_sb = const.tile([128, C * C * 128], w_dt)
        waug_sb = const.tile([kaug, C * 128], f16)
        x_sb = const.tile([kaug, T * BS], f16)
        woh_sb = const.tile([128, C2 * O], f16 if po16 else f32)
        h_g = [const.tile([128, GW], f32, name=f"h{g}") for g in range(NG)]
        eluA = [const.tile([128, GW], f16, name=f"eA{g}") for g in range(NG)]
        eluB = [const.tile([128, GW], f16, name=f"eB{g}") for g in range(NG)]
        outs_sb = const.tile([O, (T + OLAG) * BS], f16)
        if po16:
            h2f = [const.tile([128, C2 * BS], f16, name=f"h2f{p}")
                   for p in range(2)]

        nc.sync.dma_start(out=w_sb, in_=wstk[:])
        nc.sync.dma_start(out=waug_sb, in_=waug[:])
        nc.sync.dma_start(out=x_sb, in_=xaug[:])
        nc.sync.dma_start(out=woh_sb, in_=woh[:])

        if S != 1.0:
            bias_e = const.tile([128, 1], f32, name="bias_e")
            nc.vector.memset(bias_e, exp_bias)
            bias_r = const.tile([128, 1], f32, name="bias_r")
            nc.vector.memset(bias_r, 1.0 / S)

        def elu_chunk(src, g):
            """src[g] = (relu(h) - relu(1 - exp(h)))/S == elu(h)/S  (fp16).

            exp(h)/S comes from an exp-bias fold; relu(h)/S from a shadow
            h/S (DVE) that hides under the ACT exp+relu pair.
            """
            e = tmp.tile([128, GW], f16, tag="e", name="e")
            nc.scalar.activation(e, h_g[g], mybir.ActivationFunctionType.Exp,
                                 bias=(0.0 if S == 1.0 else bias_e))
            em = tmp.tile([128, GW], f16, tag="em", name="em")
            nc.scalar.activation(em, e, mybir.ActivationFunctionType.Relu,
                                 scale=-1.0,
                                 bias=(1.0 if S == 1.0 else bias_r))
            if S == 1.0:
                hin = h_g[g]
            else:
                hin = tmp.tile([128, GW], f16, tag="hs", name="hs")
                nc.vector.tensor_scalar_mul(hin, h_g[g], 1.0 / S)
            nc.vector.scalar_tensor_tensor(
                out=src[g], in0=hin, scalar=0.0, in1=em,
                op0=mybir.AluOpType.max, op1=mybir.AluOpType.subtract,
            )

        def echunk(bufs, j):
            """AP for elu chunk j out of per-group tiles."""
            return bufs[j // CPG][:, bass.ts(j % CPG, BS)]

        # j-chunk issue order inside each i group: aug first (its rhs never
        # stalls), last-computed elu chunks (C-2, C-1) deferred to the end so
        # the previous step's tail vector chain gets extra slots to finish.
        J_ORDER = list(range(C - 2)) + [C - 2, C - 1]

        def step(u, elu_in, elu_out, par=0):
            """One time step. u: step index (python int or ScalarValue expr).

            The out-projection emitted here is for step u-1 (one-step
            software pipeline): its h2f operand is long ready, so the
            in-order PE queue never stalls on it.  Column u of outs_sb
            receives out_{u-1}; column 0 is a dummy the host drops.
            """
            ps_g = [psum_p.tile([128, GW], f32, tag=f"ps{g}", name=f"ps{g}",
                                bufs=(pstail if g >= NG - ntail else None))
                    for g in range(NG)]
            xs = x_sb[:, bass.ts(u, BS)]

            po = psum_o.tile([O, BS], f32, tag="po", name="po")
            for c4 in range(C2):
                c = C2 + c4
                rhs = (h2f[par][:, bass.ts(c4, BS)] if po16 else
                       h_g[c // CPG][:, bass.ts(c % CPG, BS)])
                nc.tensor.matmul(
                    po, woh_sb[:, bass.ts(c4, O)], rhs,
                    start=(c4 == 0), stop=(c4 == C2 - 1),
                )
            if out_eng == "act":
                nc.scalar.copy(outs_sb[:, bass.ts(u, BS)], po)
            else:
                nc.vector.tensor_copy(outs_sb[:, bass.ts(u, BS)], po)

            for i in range(C):
                out_sl = ps_g[i // CPG][:, bass.ts(i % CPG, BS)]
                nc.tensor.matmul(
                    out_sl, waug_sb[:, bass.ts(i, 128)], xs,
                    start=True, stop=False,
                )
                for jn, j in enumerate(J_ORDER):
                    nc.tensor.matmul(
                        out_sl,
                        w_sb[:, bass.ts(i * C + j, 128)],
                        echunk(elu_in, j),
                        start=False, stop=(jn == C - 1),
                    )
                # group complete -> state update + elu for next step
                if i % CPG == CPG - 1:
                    g = i // CPG
                    nc.vector.scalar_tensor_tensor(
                        out=h_g[g], in0=h_g[g], scalar=1.0 - K_LEAK, in1=ps_g[g],
                        op0=mybir.AluOpType.mult, op1=mybir.AluOpType.add,
                    )
                    elu_chunk(elu_out, g)
                    if po16 and g >= NG // 2:
                        nc.scalar.copy(
                            h2f[par][:, bass.ts(g - NG // 2, GW)], h_g[g])

        def whole_pass():
            if po16:
                for t_ in h2f:
                    nc.vector.memset(t_, 0.0)
            for g in range(NG):
                nc.sync.dma_start(out=h_g[g], in_=h0[:, bass.ts(g, GW)])
                elu_chunk(eluA, g)
            if nit > 1:
                hints = {"all": tuple(mybir.ALL_ENGINES),
                         "pe": (mybir.EngineType.PE,),
                         "none": ()}[hint]
                with tc.For_i(0, nit, hint_engines=hints,
                              staggered_reset=staggered) as it:
                    base = it * blk
                    for s in range(blk):
                        src, dst = (eluA, eluB) if s % 2 == 0 else (eluB, eluA)
                        step(base + s, src, dst, par=s % 2)
            else:
                for s in range(blk):
                    src, dst = (eluA, eluB) if s % 2 == 0 else (eluB, eluA)
                    step(s, src, dst, par=s % 2)
            # drain the pipelined out-projection of the last OLAG steps
            for d in range(OLAG):
                u = T + d
                po = psum_o.tile([O, BS], f32, tag="po", name="po")
                for c4 in range(C2):
                    c = C2 + c4
                    rhs = (h2f[u % 2][:, bass.ts(c4, BS)] if po16 else
                           h_g[c // CPG][:, bass.ts(c % CPG, BS)])
                    nc.tensor.matmul(
                        po, woh_sb[:, bass.ts(c4, O)], rhs,
                        start=(c4 == 0), stop=(c4 == C2 - 1),
                    )
                nc.scalar.copy(outs_sb[:, bass.ts(u, BS)], po)

        if reps > 1:
            with tc.For_i(0, reps):
                whole_pass()
        else:
            whole_pass()

        for g in range(NG):
            nc.sync.dma_start(out=hfd[:, bass.ts(g, GW)], in_=h_g[g])
        nc.sync.dma_start(out=outsd[:], in_=outs_sb)

    nc.compile()
    return nc


def _get_module(T, blk, reps=1, ng=4, out_eng="act", staggered=False,
                wdt="f16", kaug=128, po16=True, hint="none", tmpbufs=8,
                pstail=2, ntail=1):
    key = (T, blk, reps, ng, out_eng, staggered, wdt, kaug, po16, hint,
           tmpbufs, pstail, ntail)
    if key not in _MODULE_CACHE:
        _MODULE_CACHE[key] = _build_module(T, blk, reps, ng, out_eng,
                                           staggered, wdt, kaug, po16, hint,
                                           tmpbufs, pstail, ntail)
    return _MODULE_CACHE[key]


def _prep_core_inputs(x_core, W_hh, W_hi, W_oh, b, hidden_init, T, wdt="f16",
                      kaug=128, po16=True):
    """Host-side packing for one core. x_core: [BS, T, F] float32."""
    import ml_dtypes
    from concourse import mybir as _mb
    w_dt, S = _WDT[wdt]
    np_wdt = _mb.dt.np(w_dt)
    k = np.float32(K_LEAK)
    Wp = (S * k * W_hh).astype(np_wdt)               # [H, H] scaled (k folded)
    # wstk[p, (i*C+j)*128 + m] = Wp[i*128+m, j*128+p]
    W4 = Wp.reshape(C, 128, C, 128)                  # [i, m, j, p]
    wstk = np.ascontiguousarray(W4.transpose(3, 0, 2, 1).reshape(128, C * C * 128))

    W_him = W_hi.copy()
    W_him[H // 2:, :] = 0.0
    waug = np.zeros((kaug, C * 128), np.float16)
    waug[:F, :] = (k * W_him).T.astype(np.float16)   # [F, H]
    waug[F, :] = (k * b).astype(np.float16)

    xa = np.zeros((kaug, T, BS), np.float32)
    xa[:F] = x_core.transpose(2, 1, 0)               # [F, T, BS]
    xa[F] = 1.0                                      # constant-one row (bias)
    xaug = xa.reshape(kaug, T * BS).astype(np.float16)

    # woh[p, c4*O + o] = W_oh[o, (C2+c4)*128 + p]
    Woh2 = W_oh[:, H // 2:].astype(np.float16 if po16 else np.float32)
    woh = np.ascontiguousarray(
        Woh2.reshape(O, C2, 128).transpose(2, 1, 0).reshape(128, C2 * O))

    # h0T[p, c*BS + b] = hidden_init[c*128+p]
    h0 = np.repeat(hidden_init.reshape(C, 128).T[:, :, None], BS, axis=2)
    h0 = np.ascontiguousarray(h0.reshape(128, C * BS).astype(np.float32))

    return {"wstk": wstk, "waug": waug, "xaug": xaug, "woh": woh, "h0": h0}


def run(x, W_hi, W_hh, b, W_oh, hidden_init, T=T_FULL, blk=20, trace=False,
        reps=1, ng=4, out_eng="act", staggered=False, wdt="f16", kaug=128,
        po16=True, hint="none", tmpbufs=8, pstail=2, ntail=1):
    x = np.asarray(x, np.float32)
    W_hi = np.asarray(W_hi, np.float32)
    W_hh = np.asarray(W_hh, np.float32)
    b = np.asarray(b, np.float32)
    W_oh = np.asarray(W_oh, np.float32)
    hidden_init = np.asarray(hidden_init, np.float32)

    nc = _get_module(T, blk, reps, ng, out_eng, staggered, wdt, kaug, po16,
                     hint, tmpbufs, pstail, ntail)
    in_maps = [
        _prep_core_inputs(x[c * BS:(c + 1) * BS, :T], W_hh, W_hi, W_oh, b,
                          hidden_init, T, wdt, kaug, po16)
        for c in range(NCORES)
    ]
    res = run_bass_kernel_spmd(nc, in_maps, core_ids=list(range(NCORES)),
                               trace=trace)

    outs = np.empty((B, T, O), np.float32)
    h_final = np.empty((B, H), np.float32)
    for c in range(NCORES):
        olag = 2 if po16 else 1
        om = res.results[c]["outsd"].astype(np.float32)   # [O, (T+olag)*BS]
        outs[c * BS:(c + 1) * BS] = (
            om.reshape(O, T + olag, BS)[:, olag:].transpose(2, 1, 0))
        hf = res.results[c]["hfd"]                        # [128, C*BS]
        h_final[c * BS:(c + 1) * BS] = (
            hf.reshape(128, C, BS).transpose(2, 1, 0).reshape(BS, H))
    return (outs, h_final), res


def kernel(x, W_hi, W_hh, b, W_oh, hidden_init):
    (outs, h_final), _ = run(x, W_hi, W_hh, b, W_oh, hidden_init)
    return outs, h_final
